# revision 85
# baseline (speedup 1.0000x reference)
"""Bass/TRN2 kernel for the DNC-style scatter_memory problem.

Strategy (8 NeuronCores, data-parallel over N = 1M rows):
  - Shard all N-sized tensors row-wise: core c gets rows [c*R, (c+1)*R), R = N/8.
    On-chip layout: SBUF partition p owns rows [p*L, (p+1)*L) of the shard, so
    every DMA moves large contiguous per-partition blocks at full rate, and
    per-row reductions become segmented ops along the free dimension.
  - Schedule (the stream is DVE/DMA co-paced at ~95-105us/core, then a ~15us
    AllGather for the softmax denominator, with everything else hidden):
      * The memory chunks stream back-to-back on the SP HWDGE queue; the
        N-sized side tensors (read_weighting, prev usage/ww/precedence) stream
        AFTER the last mem chunk so they never delay the denominator D, and
        their processing hides inside the collective window.
      * DVE runs the custom scans (dot with the write key as a prefix-sum of
        products against a 512-wide broadcast wk pattern; sum-of-squares as a
        two-stream half-row scan) plus the bulk-epilogue diffs/q in its idle
        slots.  For 6 of the 16 chunks the sum-of-squares is offloaded to
        ScalarE (Square) + a GpSimd pairwise tree, keeping DVE (~91us busy)
        at the DMA pace.
      * ScalarE builds the broadcast pattern tiles (partition-replicating
        DMA seeds + doublings), squares the offloaded chunks in quarters,
        gathers the scan row-ends (issued one chunk late, high priority, so
        they neither stall the scans' scratch rotation nor queue behind a
        square), does rsqrt via exp(-0.5*ln) (single act-table set), the
        softmax exp with fused row-sum accumulation, and the ww scaling.
      * GpSimd (Pool) runs the offloaded sum-of-squares trees, the last
        chunk's piece epilogues, and two retention tree levels.
      * Retention phi = prod_r(1 - w_r*f_r) uses a fused DVE op (1 - a*b)
        plus a pairwise tree as single strided ops (level 1 DVE, levels 2-3
        Pool), pipelined per rw eighth in the collective shadow; usage and
        the us output land before the gather returns.
      * D = sum(E) combines across cores with an in-kernel AllGather of the 8
        per-core partials (d_loc written via the gpsimd SWDGE; the last input
        transfer is gated behind it with a tiny SP-queue read so the DMA
        engines are clear the instant d_loc is ready).  The post-collective
        tail is a half-wise Act->DVE->DMA pipeline with ww on the Act queue
        and new_prec on the SP queue.
  - The sort+cumprod allocation weighting: usage is in [0,1], so the ascending
    exclusive cumprod underflows to exactly 0.0 in fp32 after a handful of
    terms; only the few smallest usage entries have nonzero alloc. The host
    finds the K smallest usage values (from the usage output we must produce
    anyway), replays the fp32 cumprod exactly, and sparsely adds wg*ag*alloc
    into ww/new_prec. sum(ww) equals wg to ~1e-7 (the softmax sums to 1 and
    sum(alloc) telescopes to 1 - prod(usage) = 1 in fp32), which the device
    uses for the precedence update.
"""

import numpy as np

N_FULL = 1048576
W = 64
RH = 8
NCORES = 8
R = N_FULL // NCORES          # 131072 rows per core
P = 128
L = R // P                    # 1024 rows per SBUF partition
NCH = 16                      # chunks per core
LCH = L // NCH                # 64 rows per partition per chunk
FCH = LCH * W                 # 4096 memory floats per partition per chunk
FRW = LCH * RH                # 512 read_weighting floats per partition per chunk
EPS = 1e-8

OFF = (2, 4, 6, 8, 10, 12)   # chunks whose sum-of-squares runs on Act+Pool

_CACHE = {}


def _register_ops():
    """Register custom DVE ops at runtime (one fused 1x-rate pass each)."""
    if "ops" in _CACHE:
        return _CACHE["ops"]
    from concourse.dve_ops import OPS, DveOp, _SUB_OPCODE_FOR_NAME, _CUSTOM_DVE_ROW_BASE
    from concourse.dve_spec import (
        Spec, Src0, Src1, scan, sq, AluOp, lower, One, _has_src1,
    )
    from concourse.dve_uop import DveOpSpec

    def reg(name, spec):
        for op in OPS:
            if op.name == name:
                return op
        row = _CUSTOM_DVE_ROW_BASE + len(OPS)
        assert row < 0x20, "OPS overflow"
        _SUB_OPCODE_FOR_NAME[name] = row
        s = DveOpSpec(name=name, opcode=row, uops=lower(spec, ver="v3"),
                      rd1_en=_has_src1(spec))
        op = DveOp(name, spec, subdim=False, uops_sha={"v3": s.sha("v3")})
        OPS.append(op)
        return op

    def _cs(f):
        return lambda in0, in1: np.cumsum(
            f(in0.reshape(in0.shape[0], -1).astype(np.float32),
              in1.reshape(in1.shape[0], -1).astype(np.float32)),
            axis=-1, dtype=np.float32)

    ops = {
        "muladd_scan": reg("ANT_MULADD_SCAN", Spec(
            body=scan(AluOp.ADD, Src0 * Src1),
            reference=_cs(lambda a, b: a * b))),
        "sqsum_scan": reg("ANT_SQSUM_SCAN", Spec(
            body=scan(AluOp.ADD, sq(Src0) + sq(Src1)),
            reference=_cs(lambda a, b: a * a + b * b))),
        "union_gate": reg("ANT_UNION_GATE", Spec(
            body=Src0 + Src1 - Src0 * Src1,
            reference=lambda in0, in1: (in0 + in1 - in0 * in1).astype(np.float32))),
        "one_minus_mul": reg("ANT_ONE_MINUS_MUL", Spec(
            body=One - Src0 * Src1,
            reference=lambda in0, in1: (1.0 - in0 * in1).astype(np.float32))),
    }
    _CACHE["ops"] = ops
    return ops


def _build(nreps=1):
    import concourse.bacc as bacc
    import concourse.mybir as mybir
    from concourse.tile import TileContext

    ops = _register_ops()
    F32 = mybir.dt.float32
    Alu = mybir.AluOpType
    Act = mybir.ActivationFunctionType
    AX = mybir.AxisListType.X

    nc = bacc.Bacc("TRN2", target_bir_lowering=False, debug=False,
                   num_devices=NCORES)

    try:
        from concourse.hw_specs import get_activation_tables
        ACT_SET_LN_EXP = list(get_activation_tables(nc.m.arch)).index(
            "natural_log_exp_and_others")
    except Exception:
        ACT_SET_LN_EXP = None  # fall back to auto-inserted table loads

    mem = nc.declare_dram_parameter("mem", [R, W], F32, isOutput=False)
    rw = nc.declare_dram_parameter("rw", [R, RH], F32, isOutput=False)
    pu = nc.declare_dram_parameter("pu", [R], F32, isOutput=False)
    pw = nc.declare_dram_parameter("pw", [R], F32, isOutput=False)
    prec = nc.declare_dram_parameter("prec", [R], F32, isOutput=False)
    wk = nc.declare_dram_parameter("wk", [W], F32, isOutput=False)
    scal = nc.declare_dram_parameter("scal", [3], F32, isOutput=False)  # beta, ag, wg
    fgrep = nc.declare_dram_parameter("fgrep", [FRW], F32, isOutput=False)
    o_ww = nc.declare_dram_parameter("o_ww", [R], F32, isOutput=True)
    o_us = nc.declare_dram_parameter("o_us", [R], F32, isOutput=True)
    o_np = nc.declare_dram_parameter("o_np", [R], F32, isOutput=True)

    d_locB = nc.dram_tensor("d_locB", [1, 1], F32)
    d_gathB = nc.dram_tensor("d_gathB", [1, NCORES], F32, addr_space="Shared")
    d_scr = nc.dram_tensor("d_scr", [1, 1], F32)

    memf = mem.ap().rearrange("(p l) w -> p (l w)", p=P)
    rwf = rw.ap().rearrange("(p l) h -> p (l h)", p=P)
    puf = pu.ap().rearrange("(p l) -> p l", p=P)
    pwf = pw.ap().rearrange("(p l) -> p l", p=P)
    precf = prec.ap().rearrange("(p l) -> p l", p=P)
    wwf = o_ww.ap().rearrange("(p l) -> p l", p=P)
    usf = o_us.ap().rearrange("(p l) -> p l", p=P)
    npf = o_np.ap().rearrange("(p l) -> p l", p=P)

    # chunk-0 pieces (floats per partition): small first pieces for an early
    # DVE start; scan restarts at rows 8, 16, 32, 48
    C0P = (512, 512, 1024, 1024, 1024)
    # chunk-15 pieces: 4x 1024 floats (16 rows each)
    NLP = 4
    FLP = FCH // NLP              # 1024 floats
    LLP = LCH // NLP              # 16 rows
    NE = 8                        # rw eighths
    FE8 = (L * RH) // NE          # 1024 rw floats per eighth

    with TileContext(nc) as tc:
        for _rep in range(nreps):
            with (
                tc.tile_pool(name="const", bufs=1) as cpool,
                tc.tile_pool(name="full", bufs=1) as fpool,
                tc.tile_pool(name="x", bufs=3) as xpool,
                tc.tile_pool(name="sq", bufs=2) as sqpool,
                tc.tile_pool(name="sc", bufs=2) as scpool,
                tc.tile_pool(name="sc2", bufs=2) as sc2pool,
                tc.tile_pool(name="ps", bufs=1, space="PSUM") as pspool,
            ):
                # Load the combined ln/exp/square act table once; the fixpoint
                # pass then inserts no per-activation reloads.
                if ACT_SET_LN_EXP is not None:
                    nc.scalar.add_instruction(mybir.InstLoadActFuncSet(
                        name=nc.get_next_instruction_name(),
                        act_func_set_id=ACT_SET_LN_EXP, ins=[], outs=[]))

                # ---------- prologue ----------
                # Chunk 0's pieces lead the SP queue; pattern tiles are built
                # from the tiny wk/fg vectors by PE broadcast + Act doublings
                # so the first muladd piece (needs WKREP[0:512]) is never
                # stalled.  Chunk 0's sqsum pieces don't need WKREP at all.
                X0 = xpool.tile([P, FCH], F32, tag="X")
                off = 0
                for fp in C0P:
                    nc.sync.dma_start(out=X0[:, off:off + fp],
                                      in_=memf[:, off:off + fp])
                    off += fp
                rw_full = fpool.tile([P, L * RH], F32)
                wk_s = cpool.tile([1, W], F32)
                nc.scalar.dma_start(out=wk_s[:, :], in_=wk.ap().rearrange("(o w) -> o w", o=1))
                sc_s = cpool.tile([1, 3], F32)
                nc.scalar.dma_start(out=sc_s[:, :], in_=scal.ap().rearrange("(o w) -> o w", o=1))

                ones_row = cpool.tile([1, P], F32)
                nc.gpsimd.memset(ones_row[:, :], 1.0)

                # pattern seeds land replicated across partitions straight
                # from DRAM (stride-0 partition dim), then Act doublings
                WKREP = cpool.tile([P, FRW], F32)   # 512; scans broadcast it
                nc.scalar.dma_start(out=WKREP[:, 0:W], in_=wk.ap().rearrange(
                    "(o w) -> o w", o=1).broadcast_to([P, W]))
                for n in (W, 2 * W, 4 * W):   # -> 512
                    nc.scalar.copy(WKREP[:, n:2 * n], WKREP[:, 0:n])
                FGREP = cpool.tile([P, FE8], F32)   # 1024 = one rw eighth
                nc.scalar.dma_start(out=FGREP[:, 0:RH], in_=fgrep.ap()[0:RH].rearrange(
                    "(o f) -> o f", o=1).broadcast_to([P, RH]))
                for n in (RH, 2 * RH, 4 * RH, 8 * RH, 16 * RH, 32 * RH, 64 * RH):
                    nc.scalar.copy(FGREP[:, n:2 * n], FGREP[:, 0:n])

                # small-scalar tiles (computed on Act/Pool/PE so the DVE scan
                # stream is never interrupted)
                wk2 = cpool.tile([1, W], F32)
                kw2 = cpool.tile([1, 1], F32)
                ky = cpool.tile([1, 1], F32)
                brk = cpool.tile([1, 1], F32)   # beta / ||wk||
                ag1 = cpool.tile([1, 1], F32)   # wg * (1 - ag)
                T = cpool.tile([1, 1], F32)     # 1 - wg
                brk_ps = pspool.tile([P, 1], F32)
                brk_bc = cpool.tile([P, 1], F32)
                T_ps = pspool.tile([P, 1], F32)
                T_bc = cpool.tile([P, 1], F32)
                ag_ps = pspool.tile([P, 1], F32)
                ag_bc = cpool.tile([P, 1], F32)

                def small_scalars():
                    # beta/||wk|| via rsqrt(x) = exp(-0.5*ln(x)); wg*(1-ag);
                    # 1-wg; per-partition broadcasts via PE.  kw2 = sum(wk^2)
                    # uses the Act accumulate path (DVE stays scan-only).
                    nc.gpsimd.tensor_tensor(wk2[:, :], wk_s[:, :], wk_s[:, :], op=Alu.mult)
                    nc.scalar.activation(wk2[:, :], wk2[:, :], Act.Copy,
                                         accum_out=kw2[:, :])
                    nc.scalar.activation(ky[:, :], kw2[:, :], Act.Ln)
                    nc.scalar.activation(ky[:, :], ky[:, :], Act.Exp, scale=-0.5)
                    nc.gpsimd.tensor_tensor(brk[:, :], sc_s[:, 0:1], ky[:, :], op=Alu.mult)
                    nc.gpsimd.tensor_scalar(ag1[:, :], sc_s[:, 1:2], -1.0, 1.0,
                                            op0=Alu.mult, op1=Alu.add)
                    nc.gpsimd.tensor_tensor(ag1[:, :], ag1[:, :], sc_s[:, 2:3], op=Alu.mult)
                    nc.gpsimd.tensor_scalar(T[:, :], sc_s[:, 2:3], -1.0, 1.0,
                                            op0=Alu.mult, op1=Alu.add)
                    nc.tensor.matmul(brk_ps[:, :], ones_row[:, :], brk[:, :], start=True, stop=True)
                    nc.scalar.copy(brk_bc[:, :], brk_ps[:, :])
                    nc.tensor.matmul(T_ps[:, :], ones_row[:, :], T[:, :], start=True, stop=True)
                    nc.scalar.copy(T_bc[:, :], T_ps[:, :])
                    nc.tensor.matmul(ag_ps[:, :], ones_row[:, :], ag1[:, :], start=True, stop=True)
                    nc.scalar.copy(ag_bc[:, :], ag_ps[:, :])

                # ---------- persistent tiles ----------
                numE = fpool.tile([P, L], F32)   # muladd prefix row-ends
                ssE = fpool.tile([P, L], F32)    # sqsum prefix row-ends (scan chunks)
                num_full = fpool.tile([P, L], F32)  # per-row dot -> q -> usage
                ss_full = fpool.tile([P, L], F32)   # per-row sumsq -> rsqrt in place
                pu_full = fpool.tile([P, L], F32)
                pw_full = fpool.tile([P, L], F32)
                np_full = fpool.tile([P, L], F32)   # prec -> T*prec -> +ww
                Dp = fpool.tile([P, 7], F32)     # exp row-sum partials
                DlB = cpool.tile([1, 1], F32)
                E_full = numE   # numE[sl] is dead once num_full[sl] is diffed

                # Row-end gathers run on Act but are issued one chunk LATE so
                # they never head-of-line block a square (whose input DMA
                # lands before the lagging DVE finishes the previous scans).
                pending_gathers = []

                def flush_gathers():
                    with tc.high_priority():
                        for g in pending_gathers:
                            g()
                    pending_gathers.clear()

                def sq_scan(X, view, sl, gather_eng=None):
                    # sumsq: two-stream halves prefix-sum; row ends -> ssE
                    SC2 = sc2pool.tile([P, FCH // 2], F32, tag="SC2")
                    n2 = (view.stop - view.start) // 2
                    v0 = X[:, view].rearrange("p (l w) -> p l w", w=W)[:, :, 0:W // 2]
                    v1 = X[:, view].rearrange("p (l w) -> p l w", w=W)[:, :, W // 2:W]
                    nc.vector._custom_dve(ops["sqsum_scan"], out=SC2[:, 0:n2],
                                          in0=v0, in1=v1)
                    e2 = SC2[:, 0:n2].rearrange("p (l h) -> p l h", h=W // 2)[:, :, W // 2 - 1:W // 2] \
                        .rearrange("p l o -> p (l o)")
                    if gather_eng is not None:
                        gather_eng.tensor_copy(ssE[:, sl], e2[:, :])
                    else:
                        pending_gathers.append(
                            lambda e2=e2, sl=sl: nc.scalar.copy(ssE[:, sl], e2[:, :]))

                def ma_scan(X, view, sl, gather_eng=None):
                    # num: prefix-sum of m*wk; row ends -> numE
                    SC = scpool.tile([P, FCH], F32, tag="SC")
                    n = view.stop - view.start
                    if n <= FRW:
                        wk_in = WKREP[:, 0:n]
                    else:
                        wk_in = WKREP[:, :].rearrange("p (o f) -> p o f", o=1) \
                            .broadcast_to([P, n // FRW, FRW])
                    nc.vector._custom_dve(ops["muladd_scan"], out=SC[:, 0:n],
                                          in0=X[:, view], in1=wk_in)
                    ev = SC[:, 0:n].rearrange("p (l w) -> p l w", w=W)[:, :, W - 1:W] \
                        .rearrange("p l o -> p (l o)")
                    if gather_eng is not None:
                        gather_eng.tensor_copy(numE[:, sl], ev[:, :])
                    else:
                        pending_gathers.append(
                            lambda ev=ev, sl=sl: nc.scalar.copy(numE[:, sl], ev[:, :]))

                def sq_offload(X, view, sl, nq=4):
                    # sumsq on Act (square, in pieces so pending gathers can
                    # slot between) + Pool pairwise tree; the final tree level
                    # writes per-row sums directly into ss_full
                    SQ = sqpool.tile([P, FCH], F32, tag="SQ")
                    n = view.stop - view.start
                    for sQ in range(nq):
                        qv = slice(view.start + sQ * n // nq,
                                   view.start + (sQ + 1) * n // nq)
                        sv = slice(sQ * n // nq, (sQ + 1) * n // nq)
                        nc.scalar.activation(SQ[:, sv], X[:, qv], Act.Square)
                    vv = SQ[:, 0:n].rearrange("p (l w) -> p l w", w=W)
                    h = W // 2
                    while h > 1:
                        nc.gpsimd.tensor_tensor(vv[:, :, 0:h], vv[:, :, 0:h],
                                                vv[:, :, h:2 * h], op=Alu.add)
                        h //= 2
                    nc.gpsimd.tensor_tensor(
                        ss_full[:, sl],
                        vv[:, :, 0:1].rearrange("p l o -> p (l o)"),
                        vv[:, :, 1:2].rearrange("p l o -> p (l o)"),
                        op=Alu.add)

                def seg_diff(dst, src, lo, hi, eng=None):
                    # dst[lo+1:hi] = diff(src); dst[lo] = src[lo]
                    eng = eng or nc.gpsimd
                    eng.tensor_tensor(dst[:, lo + 1:hi], src[:, lo + 1:hi],
                                      src[:, lo:hi - 1], op=Alu.subtract)
                    eng.tensor_copy(dst[:, lo:lo + 1], src[:, lo:lo + 1])

                def rsqrt_q_exp(sl, dp_col, qeng=None):
                    # ss_full -> rsqrt in place (Act); q = num*rsqrt in place;
                    # E = exp(brk*q) with fused row-sum accum (Act)
                    qeng = qeng or nc.gpsimd
                    nc.scalar.activation(ss_full[:, sl], ss_full[:, sl], Act.Ln)
                    nc.scalar.activation(ss_full[:, sl], ss_full[:, sl], Act.Exp,
                                         scale=-0.5)
                    qeng.tensor_tensor(num_full[:, sl], num_full[:, sl],
                                       ss_full[:, sl], op=Alu.mult)
                    nc.scalar.activation(E_full[:, sl], num_full[:, sl], Act.Exp,
                                         scale=brk_bc[:, :], accum_out=Dp[:, dp_col:dp_col + 1])

                def c0_fixups():
                    # chunk-0 num: diff the whole chunk then re-copy raw
                    # prefix-ends at the piece-restart rows {8,16} and {32,48}
                    # (ssq came from the offload tree — already direct values)
                    for dst, src in ((num_full, numE),):
                        seg_diff(dst, src, 0, LCH, eng=nc.vector)
                        for st, cnt, step in ((8, 2, 8), (32, 2, 16)):
                            sv = src[:, st:st + cnt * step].rearrange(
                                "p (c l) -> p c l", l=step)[:, :, 0:1].rearrange("p c o -> p (c o)")
                            dv = dst[:, st:st + cnt * step].rearrange(
                                "p (c l) -> p c l", l=step)[:, :, 0:1].rearrange("p c o -> p (c o)")
                            nc.vector.tensor_copy(dv, sv)

                def bulk_half(h):
                    # epilogue for chunks 1..7 (h=0, incl chunk 0 rsqrt) /
                    # 8..13 (h=1): num diffs (one big diff + strided raw
                    # chunk-start copies), ssq diffs for scan chunks only
                    # (offload chunks hold direct values), then rsqrt/q/exp.
                    lo = LCH if h == 0 else 8 * LCH
                    hi = 8 * LCH if h == 0 else 14 * LCH
                    nc.vector.tensor_tensor(num_full[:, lo + 1:hi], numE[:, lo + 1:hi],
                                            numE[:, lo:hi - 1], op=Alu.subtract)
                    nE = numE[:, lo:hi].rearrange("p (c l) -> p c l", l=LCH)[:, :, 0:1] \
                        .rearrange("p c o -> p (c o)")
                    nF = num_full[:, lo:hi].rearrange("p (c l) -> p c l", l=LCH)[:, :, 0:1] \
                        .rearrange("p c o -> p (c o)")
                    nc.vector.tensor_copy(nF, nE)
                    for c in range(1 if h == 0 else 8, 8 if h == 0 else 14):
                        if c in OFF or c == 1:
                            continue
                        seg_diff(ss_full, ssE, c * LCH, (c + 1) * LCH, eng=nc.vector)
                    # half-restart raw prefix-ends at rows c*64+32 of the
                    # scan chunks: one strided copy per array (ssq skips
                    # chunk 1, whose values came direct from the tree)
                    base = 96 if h == 0 else 608
                    cnt = 4 if h == 0 else 3
                    for dst, src, b2, c2 in ((ss_full, ssE, base + (2 * LCH if h == 0 else 0),
                                              cnt - (1 if h == 0 else 0)),
                                             (num_full, numE, base, cnt)):
                        sv = src[:, b2:b2 + c2 * 2 * LCH].rearrange(
                            "p (c l) -> p c l", l=2 * LCH)[:, :, 0:1].rearrange("p c o -> p (c o)")
                        dv = dst[:, b2:b2 + c2 * 2 * LCH].rearrange(
                            "p (c l) -> p c l", l=2 * LCH)[:, :, 0:1].rearrange("p c o -> p (c o)")
                        nc.vector.tensor_copy(dv, sv)
                    rsqrt_q_exp(slice(0 if h == 0 else lo, hi), h, qeng=nc.vector)

                # ---------- chunk 0: ssq offloaded (Act/Pool are idle this
                # early), muladd per piece on DVE ----------
                off = 0
                for i, fp in enumerate(C0P):
                    flush_gathers()
                    view = slice(off, off + fp)
                    sl = slice(off // W, (off + fp) // W)
                    sq_offload(X0, view, sl, nq=1)
                    ma_scan(X0, view, sl)
                    off += fp

                # ---------- chunks 1..14 (scan chunks in DMA/scan halves so
                # DVE starts each chunk ~2.9us before its full land) ---------
                for c in range(1, NCH - 1):
                    X = xpool.tile([P, FCH], F32, tag="X")
                    if c in OFF:
                        nc.sync.dma_start(out=X[:, :], in_=memf[:, c * FCH:(c + 1) * FCH])
                    else:
                        nc.sync.dma_start(out=X[:, 0:FCH // 2],
                                          in_=memf[:, c * FCH:c * FCH + FCH // 2])
                        nc.sync.dma_start(out=X[:, FCH // 2:FCH],
                                          in_=memf[:, c * FCH + FCH // 2:(c + 1) * FCH])

                    # lag-1 gathers' DVE deps are a full chunk old — flush
                    # first so they never queue behind this chunk's square
                    flush_gathers()
                    if c in OFF:
                        sq_offload(X, slice(0, FCH), slice(c * LCH, (c + 1) * LCH))
                        ma_scan(X, slice(0, FCH), slice(c * LCH, (c + 1) * LCH))
                    else:
                        if c == 1:
                            # chunk 1's ssq rides Act/Pool too (both idle early)
                            sq_offload(X, slice(0, FCH), slice(LCH, 2 * LCH))
                        # half-scans (restart at row c*64+32)
                        for hh in range(2):
                            hf = slice(hh * FCH // 2, (hh + 1) * FCH // 2)
                            hl = slice(c * LCH + hh * LCH // 2,
                                       c * LCH + (hh + 1) * LCH // 2)
                            if c != 1:
                                sq_scan(X, hf, hl)
                            ma_scan(X, hf, hl)

                    if c == 2:
                        small_scalars()
                    if c == 3:
                        c0_fixups()
                    if c == 7:
                        flush_gathers()
                        bulk_half(0)
                    if c == 13:
                        flush_gathers()
                        bulk_half(1)
                    if c == NCH - 2:
                        # chunk 14 solo epilogue
                        flush_gathers()
                        with tc.high_priority():
                            seg_diff(ss_full, ssE, 14 * LCH, 15 * LCH)
                            seg_diff(num_full, numE, 14 * LCH, 15 * LCH)
                            h32 = 14 * LCH + LCH // 2
                            nc.gpsimd.tensor_copy(ss_full[:, h32:h32 + 1], ssE[:, h32:h32 + 1])
                            nc.gpsimd.tensor_copy(num_full[:, h32:h32 + 1], numE[:, h32:h32 + 1])
                            rsqrt_q_exp(slice(14 * LCH, 15 * LCH), 2)

                # ---------- last chunk: 4 pieces, pipelined mini-epilogue ---
                XL = xpool.tile([P, FCH], F32, tag="X")
                base = (NCH - 1) * FCH
                offd = 0
                for fp in (1024, 1024, 1536, 512):
                    nc.sync.dma_start(out=XL[:, offd:offd + fp],
                                      in_=memf[:, base + offd:base + offd + fp])
                    offd += fp
                lb = (NCH - 1) * LCH
                CLP = (1024, 1024, 1536, 512)   # last piece tiny: its
                off15 = 0                        # epilogue gates the gather
                for s in range(NLP):
                    view = slice(off15, off15 + CLP[s])
                    sl = slice(lb + off15 // W, lb + (off15 + CLP[s]) // W)
                    off15 += CLP[s]
                    sq_scan(XL, view, sl, gather_eng=nc.vector)
                    ma_scan(XL, view, sl, gather_eng=nc.vector)
                    if s == NLP - 1:
                        # value-preserving touch of FGREP: every retention op
                        # reads it, so this pins all retention work AFTER the
                        # piece scans in the DVE program (the scheduler
                        # ignores priorities; only real deps order it)
                        nc.vector.tensor_scalar(FGREP[:, 0:1], FGREP[:, 0:1],
                                                1.0, 0.0, op0=Alu.mult, op1=Alu.add)
                    flush_gathers()
                    with tc.high_priority():
                        # the piece mini-epilogue is the D critical path:
                        # gathers + diffs run on DVE right after the scans
                        # (zero-hop), then Act/Pool finish the chain
                        seg_diff(ss_full, ssE, sl.start, sl.stop, eng=nc.vector)
                        seg_diff(num_full, numE, sl.start, sl.stop, eng=nc.vector)
                        rsqrt_q_exp(sl, 3 + s)

                # ---------- D partial -> AllGather (gpsimd SWDGE for d_loc so
                # it never queues behind the congested Act/SP DGE rings) -----
                with tc.high_priority():
                    nc.gpsimd.tensor_reduce(DlB[:, :], Dp[:, :],
                                            axis=mybir.AxisListType.XYZWC, op=Alu.add)
                    nc.gpsimd.dma_start(out=d_locB.ap(), in_=DlB[:, :])
                    nc.gpsimd.collective_compute(
                        "AllGather", Alu.bypass, replica_groups=[list(range(NCORES))],
                        ins=[d_locB.ap()], outs=[d_gathB.ap()])

                # rw/pu/pw/prec ride the SP queue behind the mem stream;
                # issued after the D section so the tiny d_loc write wins the
                # DMA-engine arbitration instead of queueing behind them
                for e in range(NE):
                    nc.sync.dma_start(out=rw_full[:, e * FE8:(e + 1) * FE8],
                                      in_=rwf[:, e * FE8:(e + 1) * FE8])
                nc.sync.dma_start(out=pu_full[:, :], in_=puf)
                nc.sync.dma_start(out=pw_full[:, :], in_=pwf)
                # gate the last input transfer behind the d_loc write: the SP
                # queue is in-order, so this tiny read keeps the DMA engines
                # clear of bulk traffic at the instant d_loc becomes ready,
                # letting the collective start ~2us earlier
                gate_t = cpool.tile([1, 1], F32)
                nc.sync.dma_start(out=gate_t[:, :], in_=d_locB.ap())
                nc.sync.dma_start(out=np_full[:, :], in_=precf)

                # ---------- retention + usage in the collective shadow ------
                # phi = prod_h (1 - rw*fg): fused DVE op per rw eighth, then a
                # pairwise tree as single strided ops (level 1 on DVE, levels
                # 2-3 on Pool), pipelined per eighth behind the rw DMAs
                rh_v = rw_full[:, :].rearrange("p (l h) -> p l h", h=RH)
                v2 = rw_full[:, :].rearrange("p (x t) -> p x t", t=2)
                v4 = rw_full[:, :].rearrange("p (x t) -> p x t", t=4)

                def flat(ap):
                    return ap.rearrange("p x o -> p (x o)")

                # new_prec partial: T*prec in place (runs as soon as prec lands)
                nc.scalar.activation(np_full[:, :], np_full[:, :], Act.Copy,
                                     scale=T_bc[:, :])
                ug_full = num_full  # dead after the piece epilogues
                X2 = (L * RH) // NE // 2    # v2 x-units per eighth (512)
                X4 = X2 // 2
                X8 = X4 // 2
                for e in range(NE):
                    es = slice(e * FE8, (e + 1) * FE8)
                    nc.vector._custom_dve(ops["one_minus_mul"], out=rw_full[:, es],
                                          in0=rw_full[:, es], in1=FGREP[:, :])
                    s2 = slice(e * X2, (e + 1) * X2)
                    nc.vector.tensor_tensor(flat(v2[:, s2, 0:1]), flat(v2[:, s2, 0:1]),
                                            flat(v2[:, s2, 1:2]), op=Alu.mult)
                    s4 = slice(e * X4, (e + 1) * X4)
                    nc.gpsimd.tensor_tensor(flat(v4[:, s4, 0:1]), flat(v4[:, s4, 0:1]),
                                            flat(v4[:, s4, 2:3]), op=Alu.mult)
                    s8 = slice(e * X8, (e + 1) * X8)
                    nc.gpsimd.tensor_tensor(flat(rh_v[:, s8, 0:1]), flat(rh_v[:, s8, 0:1]),
                                            flat(rh_v[:, s8, 4:5]), op=Alu.mult)
                    if e == 4:
                        # union gate only needs pu/pw; issue mid-stream so DVE
                        # reaches usage right as the last eighth finishes
                        nc.vector._custom_dve(ops["union_gate"], out=ug_full[:, :],
                                              in0=pu_full[:, :], in1=pw_full[:, :])
                ret_v = rh_v[:, :, 0:1].rearrange("p l o -> p (l o)")
                nc.vector.tensor_tensor(ug_full[:, :], ug_full[:, :], ret_v,
                                        op=Alu.mult)   # usage, in place
                nc.sync.dma_start(out=usf, in_=ug_full[:, :])

                # ---------- post-collective tail: quarter pipeline ----------
                with tc.high_priority():
                    Dg8 = cpool.tile([P, NCORES], F32)
                    nc.sync.dma_start(out=Dg8[:, :],
                                      in_=d_gathB.ap().broadcast_to([P, NCORES]))
                    Dg_bc = cpool.tile([P, 1], F32)
                    nc.vector.tensor_reduce(Dg_bc[:, :], Dg8[:, :], axis=AX, op=Alu.add)
                    B_bc = cpool.tile([P, 1], F32)
                    nc.vector.reciprocal(B_bc[:, :], Dg_bc[:, :])
                    nc.vector.tensor_tensor(B_bc[:, :], B_bc[:, :], ag_bc[:, :], op=Alu.mult)

                    ww_full = ssE  # dead after the diffs; reuse as scratch
                    # pre-arm the Act DGE queue during the collective so the
                    # first ww descriptor is at the head when its sem fires
                    nc.scalar.dma_start(out=d_scr.ap(), in_=DlB[:, :])
                    for q in range(2):
                        qs = slice(q * (L // 2), (q + 1) * (L // 2))
                        nc.scalar.activation(ww_full[:, qs], E_full[:, qs], Act.Copy,
                                             scale=B_bc[:, :])
                        nc.scalar.dma_start(out=wwf[:, qs], in_=ww_full[:, qs])
                        nc.vector.tensor_tensor(np_full[:, qs], np_full[:, qs],
                                                ww_full[:, qs], op=Alu.add)
                        nc.sync.dma_start(out=npf[:, qs], in_=np_full[:, qs])

    nc.compile()
    return nc


def _get_nc():
    if "nc" not in _CACHE:
        _CACHE["nc"] = _build()
    return _CACHE["nc"]


def _make_in_maps(inputs):
    mem = np.ascontiguousarray(inputs["memory"], dtype=np.float32)
    rw = np.ascontiguousarray(inputs["read_weighting"], dtype=np.float32)
    pu = np.ascontiguousarray(inputs["previous_usage"], dtype=np.float32)
    pw = np.ascontiguousarray(inputs["prev_write_weighting"], dtype=np.float32)
    prec = np.ascontiguousarray(inputs["precedence_weighting"], dtype=np.float32)
    wk = np.ascontiguousarray(inputs["write_key"], dtype=np.float32)
    fg = np.ascontiguousarray(inputs["free_gate"], dtype=np.float32)
    scal = np.array([inputs["write_strength"][0], inputs["allocation_gate"][0],
                     inputs["write_gate"][0]], dtype=np.float32)
    fgrep = np.tile(fg, FRW // RH)

    in_maps = []
    for c in range(NCORES):
        s = slice(c * R, (c + 1) * R)
        in_maps.append({
            "mem": mem[s], "rw": rw[s], "pu": pu[s], "pw": pw[s],
            "prec": prec[s], "wk": wk, "scal": scal, "fgrep": fgrep,
        })
    return in_maps


def _get_runner():
    """Jit the SPMD dispatch once per process; reuse across kernel() calls."""
    if "runner" in _CACHE:
        return _CACHE["runner"]
    import jax
    from jax.sharding import Mesh, PartitionSpec, NamedSharding
    from jax.experimental.shard_map import shard_map
    import concourse.mybir as mybir
    from concourse import bass2jax

    nc = _get_nc()
    bass2jax.install_neuronx_cc_hook()
    partition_name = nc.partition_id_tensor.name if nc.partition_id_tensor else None
    in_names, out_names, out_avals, zero_outs = [], [], [], []
    for alloc in nc.m.functions[0].allocations:
        if not isinstance(alloc, mybir.MemoryLocationSet):
            continue
        name = alloc.memorylocations[0].name
        if alloc.kind == "ExternalInput":
            if name != partition_name:
                in_names.append(name)
        elif alloc.kind == "ExternalOutput":
            shape = tuple(alloc.tensor_shape)
            dtype = mybir.dt.np(alloc.dtype)
            out_names.append(name)
            out_avals.append(jax.core.ShapedArray(shape, dtype))
            zero_outs.append(np.zeros(shape, dtype))
    n_params = len(in_names)
    all_in_names = list(in_names) + list(out_names)
    if partition_name is not None:
        all_in_names.append(partition_name)

    def _body(*args):
        operands = list(args)
        if partition_name is not None:
            operands.append(bass2jax.partition_id_tensor())
        return tuple(bass2jax._bass_exec_p.bind(
            *operands,
            out_avals=tuple(out_avals),
            in_names=tuple(all_in_names),
            out_names=tuple(out_names),
            lowering_input_output_aliases=(),
            sim_require_finite=True,
            sim_require_nnan=True,
            nc=nc,
        ))

    devices = jax.devices()[:NCORES]
    mesh = Mesh(np.asarray(devices), ("core",))
    in_specs = (PartitionSpec("core"),) * (n_params + len(out_names))
    out_specs = (PartitionSpec("core"),) * len(out_names)
    fn = jax.jit(shard_map(_body, mesh=mesh, in_specs=in_specs,
                           out_specs=out_specs, check_rep=False))
    sh = NamedSharding(mesh, PartitionSpec("core"))
    zeros_dev = [jax.device_put(
        np.zeros((NCORES * z.shape[0], *z.shape[1:]), z.dtype), sh)
        for z in zero_outs]

    def run(in_maps):
        concat_in = [np.concatenate(
            [np.asarray(in_maps[c][k]) for c in range(NCORES)], axis=0)
            for k in in_names]
        dev_in = [jax.device_put(a, sh) for a in concat_in]
        outs = fn(*dev_in, *zeros_dev)
        return {name: np.array(outs[i]) for i, name in enumerate(out_names)}

    _CACHE["runner"] = run
    return run


def _run_device(inputs):
    in_maps = _make_in_maps(inputs)
    try:
        out = _get_runner()(in_maps)
        return out["o_ww"], out["o_us"], out["o_np"]
    except Exception:
        from concourse.bass_utils import run_bass_kernel_spmd
        nc = _get_nc()
        res = run_bass_kernel_spmd(nc, in_maps, core_ids=list(range(NCORES)))
        ww = np.concatenate([res.results[c]["o_ww"] for c in range(NCORES)])
        us = np.concatenate([res.results[c]["o_us"] for c in range(NCORES)])
        npr = np.concatenate([res.results[c]["o_np"] for c in range(NCORES)])
        return ww, us, npr


def _alloc_fixup(usage, ww, npr, ag, wg):
    """Sparse allocation-weighting correction on the host (see module doc)."""
    K = 256
    while True:
        K = min(K, usage.shape[0])
        idx = np.argpartition(usage, K - 1)[:K]
        vals = usage[idx]
        srt = np.lexsort((idx, vals))   # stable: by value, then original index
        sv = vals[srt].astype(np.float32)
        si = idx[srt]
        cp = np.cumprod(sv, dtype=np.float32)
        if cp[-1] == 0.0 or K == usage.shape[0]:
            break
        K *= 4
    excl = np.empty_like(sv)
    excl[0] = np.float32(1.0)
    excl[1:] = cp[:-1]
    alloc = (np.float32(1.0) - sv) * excl
    nz = alloc != 0.0
    delta = np.float32(wg) * np.float32(ag) * alloc[nz]
    ww[si[nz]] += delta
    npr[si[nz]] += delta
    return ww, npr


def kernel(**inputs):
    ww, us, npr = _run_device(inputs)
    ag = float(np.float32(inputs["allocation_gate"][0]))
    wg = float(np.float32(inputs["write_gate"][0]))
    ww, npr = _alloc_fixup(us, ww, npr, ag, wg)
    return ww, us, npr


# revision 91
# speedup vs baseline: 1.1072x; 1.1072x over previous
"""Bass/TRN2 kernel for the DNC-style scatter_memory problem.

Strategy (8 NeuronCores, data-parallel over N = 1M rows):
  - Shard all N-sized tensors row-wise: core c gets rows [c*R, (c+1)*R), R = N/8.
    On-chip layout: SBUF partition p owns rows [p*L, (p+1)*L) of the shard, so
    every DMA moves large contiguous per-partition blocks at full rate, and
    per-row reductions become segmented ops along the free dimension.
  - Schedule (the stream is DVE/DMA co-paced at ~95-105us/core, then a ~15us
    AllGather for the softmax denominator, with everything else hidden):
      * The memory chunks stream back-to-back on the SP HWDGE queue; the
        N-sized side tensors (read_weighting, prev usage/ww/precedence) stream
        AFTER the last mem chunk so they never delay the denominator D, and
        their processing hides inside the collective window.
      * DVE runs the custom scans (dot with the write key as a prefix-sum of
        products against a 512-wide broadcast wk pattern; sum-of-squares as a
        two-stream half-row scan) plus the bulk-epilogue diffs/q in its idle
        slots.  For 6 of the 16 chunks the sum-of-squares is offloaded to
        ScalarE (Square) + a GpSimd pairwise tree, keeping DVE (~91us busy)
        at the DMA pace.
      * ScalarE builds the broadcast pattern tiles (partition-replicating
        DMA seeds + doublings), squares the offloaded chunks in quarters,
        gathers the scan row-ends (issued one chunk late, high priority, so
        they neither stall the scans' scratch rotation nor queue behind a
        square), does rsqrt via exp(-0.5*ln) (single act-table set), the
        softmax exp with fused row-sum accumulation, and the ww scaling.
      * GpSimd (Pool) runs the offloaded sum-of-squares trees, the last
        chunk's piece epilogues, and two retention tree levels.
      * Retention phi = prod_r(1 - w_r*f_r) uses a fused DVE op (1 - a*b)
        plus a pairwise tree as single strided ops (level 1 DVE, levels 2-3
        Pool), pipelined per rw eighth in the collective shadow; usage and
        the us output land before the gather returns.
      * D = sum(E) combines across cores with an in-kernel AllGather of the 8
        per-core partials (d_loc written via the gpsimd SWDGE; the last input
        transfer is gated behind it with a tiny SP-queue read so the DMA
        engines are clear the instant d_loc is ready).  The post-collective
        tail is a half-wise Act->DVE->DMA pipeline with ww on the Act queue
        and new_prec on the SP queue.
  - The sort+cumprod allocation weighting: usage is in [0,1], so the ascending
    exclusive cumprod underflows to exactly 0.0 in fp32 after a handful of
    terms; only the few smallest usage entries have nonzero alloc. The host
    finds the K smallest usage values (from the usage output we must produce
    anyway), replays the fp32 cumprod exactly, and sparsely adds wg*ag*alloc
    into ww/new_prec. sum(ww) equals wg to ~1e-7 (the softmax sums to 1 and
    sum(alloc) telescopes to 1 - prod(usage) = 1 in fp32), which the device
    uses for the precedence update.
"""

import numpy as np

N_FULL = 1048576
W = 64
RH = 8
NCORES = 8
R = N_FULL // NCORES          # 131072 rows per core
P = 128
L = R // P                    # 1024 rows per SBUF partition
NCH = 16                      # chunks per core
LCH = L // NCH                # 64 rows per partition per chunk
FCH = LCH * W                 # 4096 memory floats per partition per chunk
FRW = LCH * RH                # 512 read_weighting floats per partition per chunk
EPS = 1e-8

OFF = (2, 4, 6, 8, 10, 12)   # chunks whose sum-of-squares runs on Act+Pool

_CACHE = {}


def _register_ops():
    """Register custom DVE ops at runtime (one fused 1x-rate pass each)."""
    if "ops" in _CACHE:
        return _CACHE["ops"]
    from concourse.dve_ops import OPS, DveOp, _SUB_OPCODE_FOR_NAME, _CUSTOM_DVE_ROW_BASE
    from concourse.dve_spec import (
        Spec, Src0, Src1, scan, sq, AluOp, lower, One, _has_src1,
    )
    from concourse.dve_uop import DveOpSpec

    def reg(name, spec):
        for op in OPS:
            if op.name == name:
                return op
        row = _CUSTOM_DVE_ROW_BASE + len(OPS)
        assert row < 0x20, "OPS overflow"
        _SUB_OPCODE_FOR_NAME[name] = row
        s = DveOpSpec(name=name, opcode=row, uops=lower(spec, ver="v3"),
                      rd1_en=_has_src1(spec))
        op = DveOp(name, spec, subdim=False, uops_sha={"v3": s.sha("v3")})
        OPS.append(op)
        return op

    def _cs(f):
        return lambda in0, in1: np.cumsum(
            f(in0.reshape(in0.shape[0], -1).astype(np.float32),
              in1.reshape(in1.shape[0], -1).astype(np.float32)),
            axis=-1, dtype=np.float32)

    ops = {
        "muladd_scan": reg("ANT_MULADD_SCAN", Spec(
            body=scan(AluOp.ADD, Src0 * Src1),
            reference=_cs(lambda a, b: a * b))),
        "sqsum_scan": reg("ANT_SQSUM_SCAN", Spec(
            body=scan(AluOp.ADD, sq(Src0) + sq(Src1)),
            reference=_cs(lambda a, b: a * a + b * b))),
        "union_gate": reg("ANT_UNION_GATE", Spec(
            body=Src0 + Src1 - Src0 * Src1,
            reference=lambda in0, in1: (in0 + in1 - in0 * in1).astype(np.float32))),
        "one_minus_mul": reg("ANT_ONE_MINUS_MUL", Spec(
            body=One - Src0 * Src1,
            reference=lambda in0, in1: (1.0 - in0 * in1).astype(np.float32))),
    }
    _CACHE["ops"] = ops
    return ops


def _build(nreps=1):
    import concourse.bacc as bacc
    import concourse.mybir as mybir
    from concourse.tile import TileContext

    ops = _register_ops()
    F32 = mybir.dt.float32
    Alu = mybir.AluOpType
    Act = mybir.ActivationFunctionType
    AX = mybir.AxisListType.X

    nc = bacc.Bacc("TRN2", target_bir_lowering=False, debug=False,
                   num_devices=NCORES)

    try:
        from concourse.hw_specs import get_activation_tables
        ACT_SET_LN_EXP = list(get_activation_tables(nc.m.arch)).index(
            "natural_log_exp_and_others")
    except Exception:
        ACT_SET_LN_EXP = None  # fall back to auto-inserted table loads

    mem = nc.declare_dram_parameter("mem", [R, W], F32, isOutput=False)
    rw = nc.declare_dram_parameter("rw", [R, RH], F32, isOutput=False)
    pu = nc.declare_dram_parameter("pu", [R], F32, isOutput=False)
    pw = nc.declare_dram_parameter("pw", [R], F32, isOutput=False)
    prec = nc.declare_dram_parameter("prec", [R], F32, isOutput=False)
    wk = nc.declare_dram_parameter("wk", [W], F32, isOutput=False)
    scal = nc.declare_dram_parameter("scal", [3], F32, isOutput=False)  # beta, ag, wg
    fgrep = nc.declare_dram_parameter("fgrep", [FRW], F32, isOutput=False)
    o_ww = nc.declare_dram_parameter("o_ww", [R], F32, isOutput=True)
    o_us = nc.declare_dram_parameter("o_us", [R], F32, isOutput=True)
    o_np = nc.declare_dram_parameter("o_np", [R], F32, isOutput=True)

    d_locB = nc.dram_tensor("d_locB", [1, 1], F32)
    d_gathB = nc.dram_tensor("d_gathB", [1, NCORES], F32, addr_space="Shared")
    d_scr = nc.dram_tensor("d_scr", [1, 1], F32)

    memf = mem.ap().rearrange("(p l) w -> p (l w)", p=P)
    rwf = rw.ap().rearrange("(p l) h -> p (l h)", p=P)
    puf = pu.ap().rearrange("(p l) -> p l", p=P)
    pwf = pw.ap().rearrange("(p l) -> p l", p=P)
    precf = prec.ap().rearrange("(p l) -> p l", p=P)
    wwf = o_ww.ap().rearrange("(p l) -> p l", p=P)
    usf = o_us.ap().rearrange("(p l) -> p l", p=P)
    npf = o_np.ap().rearrange("(p l) -> p l", p=P)

    # chunk-0 pieces (floats per partition): small first pieces for an early
    # DVE start; scan restarts at rows 8, 16, 32, 48
    C0P = (512, 512, 1024, 1024, 1024)
    # chunk-15 pieces: 4x 1024 floats (16 rows each)
    NLP = 4
    FLP = FCH // NLP              # 1024 floats
    LLP = LCH // NLP              # 16 rows
    NE = 8                        # rw eighths
    FE8 = (L * RH) // NE          # 1024 rw floats per eighth

    with TileContext(nc) as tc:
        for _rep in range(nreps):
            with (
                tc.tile_pool(name="const", bufs=1) as cpool,
                tc.tile_pool(name="full", bufs=1) as fpool,
                tc.tile_pool(name="x", bufs=3) as xpool,
                tc.tile_pool(name="sq", bufs=2) as sqpool,
                tc.tile_pool(name="sc", bufs=2) as scpool,
                tc.tile_pool(name="sc2", bufs=2) as sc2pool,
                tc.tile_pool(name="ps", bufs=1, space="PSUM") as pspool,
            ):
                # Load the combined ln/exp/square act table once; the fixpoint
                # pass then inserts no per-activation reloads.
                if ACT_SET_LN_EXP is not None:
                    nc.scalar.add_instruction(mybir.InstLoadActFuncSet(
                        name=nc.get_next_instruction_name(),
                        act_func_set_id=ACT_SET_LN_EXP, ins=[], outs=[]))

                # ---------- prologue ----------
                # Chunk 0's pieces lead the SP queue; pattern tiles are built
                # from the tiny wk/fg vectors by PE broadcast + Act doublings
                # so the first muladd piece (needs WKREP[0:512]) is never
                # stalled.  Chunk 0's sqsum pieces don't need WKREP at all.
                X0 = xpool.tile([P, FCH], F32, tag="X")
                off = 0
                for fp in C0P:
                    nc.sync.dma_start(out=X0[:, off:off + fp],
                                      in_=memf[:, off:off + fp])
                    off += fp
                rw_full = fpool.tile([P, L * RH], F32)
                wk_s = cpool.tile([1, W], F32)
                nc.scalar.dma_start(out=wk_s[:, :], in_=wk.ap().rearrange("(o w) -> o w", o=1))
                sc_s = cpool.tile([1, 3], F32)
                nc.scalar.dma_start(out=sc_s[:, :], in_=scal.ap().rearrange("(o w) -> o w", o=1))

                ones_row = cpool.tile([1, P], F32)
                nc.gpsimd.memset(ones_row[:, :], 1.0)

                # pattern seeds land replicated across partitions straight
                # from DRAM (stride-0 partition dim), then Act doublings
                WKREP = cpool.tile([P, FRW], F32)   # 512; scans broadcast it
                nc.scalar.dma_start(out=WKREP[:, 0:W], in_=wk.ap().rearrange(
                    "(o w) -> o w", o=1).broadcast_to([P, W]))
                for n in (W, 2 * W, 4 * W):   # -> 512
                    nc.scalar.copy(WKREP[:, n:2 * n], WKREP[:, 0:n])
                FGREP = cpool.tile([P, FE8], F32)   # 1024 = one rw eighth
                nc.scalar.dma_start(out=FGREP[:, 0:RH], in_=fgrep.ap()[0:RH].rearrange(
                    "(o f) -> o f", o=1).broadcast_to([P, RH]))
                for n in (RH, 2 * RH, 4 * RH, 8 * RH, 16 * RH, 32 * RH, 64 * RH):
                    nc.scalar.copy(FGREP[:, n:2 * n], FGREP[:, 0:n])

                # small-scalar tiles (computed on Act/Pool/PE so the DVE scan
                # stream is never interrupted)
                wk2 = cpool.tile([1, W], F32)
                kw2 = cpool.tile([1, 1], F32)
                ky = cpool.tile([1, 1], F32)
                brk = cpool.tile([1, 1], F32)   # beta / ||wk||
                ag1 = cpool.tile([1, 1], F32)   # wg * (1 - ag)
                T = cpool.tile([1, 1], F32)     # 1 - wg
                brk_ps = pspool.tile([P, 1], F32)
                brk_bc = cpool.tile([P, 1], F32)
                T_ps = pspool.tile([P, 1], F32)
                T_bc = cpool.tile([P, 1], F32)
                ag_ps = pspool.tile([P, 1], F32)
                ag_bc = cpool.tile([P, 1], F32)

                def small_scalars():
                    # beta/||wk|| via rsqrt(x) = exp(-0.5*ln(x)); wg*(1-ag);
                    # 1-wg; per-partition broadcasts via PE.  kw2 = sum(wk^2)
                    # uses the Act accumulate path (DVE stays scan-only).
                    nc.gpsimd.tensor_tensor(wk2[:, :], wk_s[:, :], wk_s[:, :], op=Alu.mult)
                    nc.scalar.activation(wk2[:, :], wk2[:, :], Act.Copy,
                                         accum_out=kw2[:, :])
                    nc.scalar.activation(ky[:, :], kw2[:, :], Act.Ln)
                    nc.scalar.activation(ky[:, :], ky[:, :], Act.Exp, scale=-0.5)
                    nc.gpsimd.tensor_tensor(brk[:, :], sc_s[:, 0:1], ky[:, :], op=Alu.mult)
                    nc.gpsimd.tensor_scalar(ag1[:, :], sc_s[:, 1:2], -1.0, 1.0,
                                            op0=Alu.mult, op1=Alu.add)
                    nc.gpsimd.tensor_tensor(ag1[:, :], ag1[:, :], sc_s[:, 2:3], op=Alu.mult)
                    nc.gpsimd.tensor_scalar(T[:, :], sc_s[:, 2:3], -1.0, 1.0,
                                            op0=Alu.mult, op1=Alu.add)
                    nc.tensor.matmul(brk_ps[:, :], ones_row[:, :], brk[:, :], start=True, stop=True)
                    nc.scalar.copy(brk_bc[:, :], brk_ps[:, :])
                    nc.tensor.matmul(T_ps[:, :], ones_row[:, :], T[:, :], start=True, stop=True)
                    nc.scalar.copy(T_bc[:, :], T_ps[:, :])
                    nc.tensor.matmul(ag_ps[:, :], ones_row[:, :], ag1[:, :], start=True, stop=True)
                    nc.scalar.copy(ag_bc[:, :], ag_ps[:, :])

                # ---------- persistent tiles ----------
                numE = fpool.tile([P, L], F32)   # muladd prefix row-ends
                ssE = fpool.tile([P, L], F32)    # sqsum prefix row-ends (scan chunks)
                num_full = fpool.tile([P, L], F32)  # per-row dot -> q -> usage
                ss_full = fpool.tile([P, L], F32)   # per-row sumsq -> rsqrt in place
                pu_full = fpool.tile([P, L], F32)
                pw_full = fpool.tile([P, L], F32)
                np_full = fpool.tile([P, L], F32)   # prec -> T*prec -> +ww
                Dp = fpool.tile([P, 7], F32)     # exp row-sum partials
                DlB = cpool.tile([1, 1], F32)
                E_full = numE   # numE[sl] is dead once num_full[sl] is diffed

                # Row-end gathers run on Act but are issued one chunk LATE so
                # they never head-of-line block a square (whose input DMA
                # lands before the lagging DVE finishes the previous scans).
                pending_gathers = []

                def flush_gathers():
                    with tc.high_priority():
                        for g in pending_gathers:
                            g()
                    pending_gathers.clear()

                def sq_scan(X, view, sl, gather_eng=None):
                    # sumsq: two-stream halves prefix-sum; row ends -> ssE
                    SC2 = sc2pool.tile([P, FCH // 2], F32, tag="SC2")
                    n2 = (view.stop - view.start) // 2
                    v0 = X[:, view].rearrange("p (l w) -> p l w", w=W)[:, :, 0:W // 2]
                    v1 = X[:, view].rearrange("p (l w) -> p l w", w=W)[:, :, W // 2:W]
                    nc.vector._custom_dve(ops["sqsum_scan"], out=SC2[:, 0:n2],
                                          in0=v0, in1=v1)
                    e2 = SC2[:, 0:n2].rearrange("p (l h) -> p l h", h=W // 2)[:, :, W // 2 - 1:W // 2] \
                        .rearrange("p l o -> p (l o)")
                    if gather_eng is not None:
                        gather_eng.tensor_copy(ssE[:, sl], e2[:, :])
                    else:
                        pending_gathers.append(
                            lambda e2=e2, sl=sl: nc.scalar.copy(ssE[:, sl], e2[:, :]))

                def ma_scan(X, view, sl, gather_eng=None):
                    # num: prefix-sum of m*wk; row ends -> numE
                    SC = scpool.tile([P, FCH], F32, tag="SC")
                    n = view.stop - view.start
                    if n <= FRW:
                        wk_in = WKREP[:, 0:n]
                    else:
                        wk_in = WKREP[:, :].rearrange("p (o f) -> p o f", o=1) \
                            .broadcast_to([P, n // FRW, FRW])
                    nc.vector._custom_dve(ops["muladd_scan"], out=SC[:, 0:n],
                                          in0=X[:, view], in1=wk_in)
                    ev = SC[:, 0:n].rearrange("p (l w) -> p l w", w=W)[:, :, W - 1:W] \
                        .rearrange("p l o -> p (l o)")
                    if gather_eng is not None:
                        gather_eng.tensor_copy(numE[:, sl], ev[:, :])
                    else:
                        pending_gathers.append(
                            lambda ev=ev, sl=sl: nc.scalar.copy(numE[:, sl], ev[:, :]))

                def sq_offload(X, view, sl, nq=4):
                    # sumsq on Act (square, in pieces so pending gathers can
                    # slot between) + Pool pairwise tree; the final tree level
                    # writes per-row sums directly into ss_full
                    SQ = sqpool.tile([P, FCH], F32, tag="SQ")
                    n = view.stop - view.start
                    for sQ in range(nq):
                        qv = slice(view.start + sQ * n // nq,
                                   view.start + (sQ + 1) * n // nq)
                        sv = slice(sQ * n // nq, (sQ + 1) * n // nq)
                        nc.scalar.activation(SQ[:, sv], X[:, qv], Act.Square)
                    vv = SQ[:, 0:n].rearrange("p (l w) -> p l w", w=W)
                    h = W // 2
                    while h > 1:
                        nc.gpsimd.tensor_tensor(vv[:, :, 0:h], vv[:, :, 0:h],
                                                vv[:, :, h:2 * h], op=Alu.add)
                        h //= 2
                    nc.gpsimd.tensor_tensor(
                        ss_full[:, sl],
                        vv[:, :, 0:1].rearrange("p l o -> p (l o)"),
                        vv[:, :, 1:2].rearrange("p l o -> p (l o)"),
                        op=Alu.add)

                def seg_diff(dst, src, lo, hi, eng=None):
                    # dst[lo+1:hi] = diff(src); dst[lo] = src[lo]
                    eng = eng or nc.gpsimd
                    if hi > lo + 1:
                        eng.tensor_tensor(dst[:, lo + 1:hi], src[:, lo + 1:hi],
                                          src[:, lo:hi - 1], op=Alu.subtract)
                    eng.tensor_copy(dst[:, lo:lo + 1], src[:, lo:lo + 1])

                def rsqrt_q_exp(sl, dp_col, qeng=None):
                    # ss_full -> rsqrt in place (Act); q = num*rsqrt in place;
                    # E = exp(brk*q) with fused row-sum accum (Act)
                    qeng = qeng or nc.gpsimd
                    nc.scalar.activation(ss_full[:, sl], ss_full[:, sl], Act.Ln)
                    nc.scalar.activation(ss_full[:, sl], ss_full[:, sl], Act.Exp,
                                         scale=-0.5)
                    qeng.tensor_tensor(num_full[:, sl], num_full[:, sl],
                                       ss_full[:, sl], op=Alu.mult)
                    nc.scalar.activation(E_full[:, sl], num_full[:, sl], Act.Exp,
                                         scale=brk_bc[:, :], accum_out=Dp[:, dp_col:dp_col + 1])

                def c0_fixups():
                    # chunk-0 num: diff the whole chunk then re-copy raw
                    # prefix-ends at the piece-restart rows {8,16} and {32,48}
                    # (ssq came from the offload tree — already direct values)
                    for dst, src in ((num_full, numE),):
                        seg_diff(dst, src, 0, LCH, eng=nc.vector)
                        for st, cnt, step in ((8, 2, 8), (32, 2, 16)):
                            sv = src[:, st:st + cnt * step].rearrange(
                                "p (c l) -> p c l", l=step)[:, :, 0:1].rearrange("p c o -> p (c o)")
                            dv = dst[:, st:st + cnt * step].rearrange(
                                "p (c l) -> p c l", l=step)[:, :, 0:1].rearrange("p c o -> p (c o)")
                            nc.vector.tensor_copy(dv, sv)

                def bulk_half(h):
                    # epilogue for chunks 1..7 (h=0, incl chunk 0 rsqrt) /
                    # 8..13 (h=1): num diffs (one big diff + strided raw
                    # chunk-start copies), ssq diffs for scan chunks only
                    # (offload chunks hold direct values), then rsqrt/q/exp.
                    lo = LCH if h == 0 else 8 * LCH
                    hi = 8 * LCH if h == 0 else 14 * LCH
                    nc.vector.tensor_tensor(num_full[:, lo + 1:hi], numE[:, lo + 1:hi],
                                            numE[:, lo:hi - 1], op=Alu.subtract)
                    nE = numE[:, lo:hi].rearrange("p (c l) -> p c l", l=LCH)[:, :, 0:1] \
                        .rearrange("p c o -> p (c o)")
                    nF = num_full[:, lo:hi].rearrange("p (c l) -> p c l", l=LCH)[:, :, 0:1] \
                        .rearrange("p c o -> p (c o)")
                    nc.vector.tensor_copy(nF, nE)
                    for c in range(1 if h == 0 else 8, 8 if h == 0 else 14):
                        if c in OFF or c == 1:
                            continue
                        seg_diff(ss_full, ssE, c * LCH, (c + 1) * LCH, eng=nc.vector)
                    # half-restart raw prefix-ends at rows c*64+32 of the
                    # scan chunks: one strided copy per array (ssq skips
                    # chunk 1, whose values came direct from the tree)
                    base = 96 if h == 0 else 608
                    cnt = 4 if h == 0 else 3
                    for dst, src, b2, c2 in ((ss_full, ssE, base + (2 * LCH if h == 0 else 0),
                                              cnt - (1 if h == 0 else 0)),
                                             (num_full, numE, base, cnt)):
                        sv = src[:, b2:b2 + c2 * 2 * LCH].rearrange(
                            "p (c l) -> p c l", l=2 * LCH)[:, :, 0:1].rearrange("p c o -> p (c o)")
                        dv = dst[:, b2:b2 + c2 * 2 * LCH].rearrange(
                            "p (c l) -> p c l", l=2 * LCH)[:, :, 0:1].rearrange("p c o -> p (c o)")
                        nc.vector.tensor_copy(dv, sv)
                    rsqrt_q_exp(slice(0 if h == 0 else lo, hi), h, qeng=nc.vector)

                # ---------- chunk 0: ssq offloaded (Act/Pool are idle this
                # early), muladd per piece on DVE ----------
                off = 0
                for i, fp in enumerate(C0P):
                    flush_gathers()
                    view = slice(off, off + fp)
                    sl = slice(off // W, (off + fp) // W)
                    sq_offload(X0, view, sl, nq=1)
                    ma_scan(X0, view, sl)
                    off += fp

                # ---------- chunks 1..14 (scan chunks in DMA/scan halves so
                # DVE starts each chunk ~2.9us before its full land) ---------
                for c in range(1, NCH - 1):
                    X = xpool.tile([P, FCH], F32, tag="X")
                    if c in OFF:
                        nc.sync.dma_start(out=X[:, :], in_=memf[:, c * FCH:(c + 1) * FCH])
                    else:
                        nc.sync.dma_start(out=X[:, 0:FCH // 2],
                                          in_=memf[:, c * FCH:c * FCH + FCH // 2])
                        nc.sync.dma_start(out=X[:, FCH // 2:FCH],
                                          in_=memf[:, c * FCH + FCH // 2:(c + 1) * FCH])

                    # lag-1 gathers' DVE deps are a full chunk old — flush
                    # first so they never queue behind this chunk's square
                    flush_gathers()
                    if c in OFF:
                        sq_offload(X, slice(0, FCH), slice(c * LCH, (c + 1) * LCH))
                        ma_scan(X, slice(0, FCH), slice(c * LCH, (c + 1) * LCH))
                    else:
                        if c == 1:
                            # chunk 1's ssq rides Act/Pool too (both idle early)
                            sq_offload(X, slice(0, FCH), slice(LCH, 2 * LCH))
                        # half-scans (restart at row c*64+32)
                        for hh in range(2):
                            hf = slice(hh * FCH // 2, (hh + 1) * FCH // 2)
                            hl = slice(c * LCH + hh * LCH // 2,
                                       c * LCH + (hh + 1) * LCH // 2)
                            if c != 1:
                                sq_scan(X, hf, hl)
                            ma_scan(X, hf, hl)

                    if c == 2:
                        small_scalars()
                    if c == 3:
                        c0_fixups()
                    if c == 7:
                        flush_gathers()
                        bulk_half(0)
                    if c == 13:
                        flush_gathers()
                        bulk_half(1)
                    if c == NCH - 2:
                        # chunk 14 solo epilogue
                        flush_gathers()
                        with tc.high_priority():
                            seg_diff(ss_full, ssE, 14 * LCH, 15 * LCH)
                            seg_diff(num_full, numE, 14 * LCH, 15 * LCH)
                            h32 = 14 * LCH + LCH // 2
                            nc.gpsimd.tensor_copy(ss_full[:, h32:h32 + 1], ssE[:, h32:h32 + 1])
                            nc.gpsimd.tensor_copy(num_full[:, h32:h32 + 1], numE[:, h32:h32 + 1])
                            rsqrt_q_exp(slice(14 * LCH, 15 * LCH), 2)

                # ---------- last chunk: 4 pieces, pipelined mini-epilogue ---
                XL = xpool.tile([P, FCH], F32, tag="X")
                base = (NCH - 1) * FCH
                offd = 0
                for fp in (1024, 1024, 1536, 512):
                    nc.sync.dma_start(out=XL[:, offd:offd + fp],
                                      in_=memf[:, base + offd:base + offd + fp])
                    offd += fp
                lb = (NCH - 1) * LCH
                CLP = (1024, 1024, 1536, 512)   # last piece tiny: its
                off15 = 0                        # epilogue gates the gather
                for s in range(NLP):
                    view = slice(off15, off15 + CLP[s])
                    sl = slice(lb + off15 // W, lb + (off15 + CLP[s]) // W)
                    off15 += CLP[s]
                    sq_scan(XL, view, sl, gather_eng=nc.vector)
                    ma_scan(XL, view, sl, gather_eng=nc.vector)
                    if s == NLP - 1:
                        # value-preserving touch of FGREP: every retention op
                        # reads it, so this pins all retention work AFTER the
                        # piece scans in the DVE program (the scheduler
                        # ignores priorities; only real deps order it)
                        nc.vector.tensor_scalar(FGREP[:, 0:1], FGREP[:, 0:1],
                                                1.0, 0.0, op0=Alu.mult, op1=Alu.add)
                    flush_gathers()
                    with tc.high_priority():
                        # the piece mini-epilogue is the D critical path:
                        # gathers + diffs run on DVE right after the scans
                        # (zero-hop), then Act/Pool finish the chain
                        seg_diff(ss_full, ssE, sl.start, sl.stop, eng=nc.vector)
                        seg_diff(num_full, numE, sl.start, sl.stop, eng=nc.vector)
                        rsqrt_q_exp(sl, 3 + s)

                # ---------- D partial -> AllGather (gpsimd SWDGE for d_loc so
                # it never queues behind the congested Act/SP DGE rings) -----
                with tc.high_priority():
                    nc.gpsimd.tensor_reduce(DlB[:, :], Dp[:, :],
                                            axis=mybir.AxisListType.XYZWC, op=Alu.add)
                    nc.gpsimd.dma_start(out=d_locB.ap(), in_=DlB[:, :])
                    nc.gpsimd.collective_compute(
                        "AllGather", Alu.bypass, replica_groups=[list(range(NCORES))],
                        ins=[d_locB.ap()], outs=[d_gathB.ap()])

                # rw/pu/pw/prec ride the SP queue behind the mem stream;
                # issued after the D section so the tiny d_loc write wins the
                # DMA-engine arbitration instead of queueing behind them
                for e in range(NE):
                    nc.sync.dma_start(out=rw_full[:, e * FE8:(e + 1) * FE8],
                                      in_=rwf[:, e * FE8:(e + 1) * FE8])
                nc.sync.dma_start(out=pu_full[:, :], in_=puf)
                nc.sync.dma_start(out=pw_full[:, :], in_=pwf)
                # gate the last input transfer behind the d_loc write: the SP
                # queue is in-order, so this tiny read keeps the DMA engines
                # clear of bulk traffic at the instant d_loc becomes ready,
                # letting the collective start ~2us earlier
                gate_t = cpool.tile([1, 1], F32)
                nc.sync.dma_start(out=gate_t[:, :], in_=d_locB.ap())
                nc.sync.dma_start(out=np_full[:, :], in_=precf)

                # ---------- retention + usage in the collective shadow ------
                # phi = prod_h (1 - rw*fg): fused DVE op per rw eighth, then a
                # pairwise tree as single strided ops (level 1 on DVE, levels
                # 2-3 on Pool), pipelined per eighth behind the rw DMAs
                rh_v = rw_full[:, :].rearrange("p (l h) -> p l h", h=RH)
                v2 = rw_full[:, :].rearrange("p (x t) -> p x t", t=2)
                v4 = rw_full[:, :].rearrange("p (x t) -> p x t", t=4)

                def flat(ap):
                    return ap.rearrange("p x o -> p (x o)")

                # new_prec partial: T*prec in place (runs as soon as prec lands)
                nc.scalar.activation(np_full[:, :], np_full[:, :], Act.Copy,
                                     scale=T_bc[:, :])
                ug_full = num_full  # dead after the piece epilogues
                X2 = (L * RH) // NE // 2    # v2 x-units per eighth (512)
                X4 = X2 // 2
                X8 = X4 // 2
                for e in range(NE):
                    es = slice(e * FE8, (e + 1) * FE8)
                    nc.vector._custom_dve(ops["one_minus_mul"], out=rw_full[:, es],
                                          in0=rw_full[:, es], in1=FGREP[:, :])
                    s2 = slice(e * X2, (e + 1) * X2)
                    nc.vector.tensor_tensor(flat(v2[:, s2, 0:1]), flat(v2[:, s2, 0:1]),
                                            flat(v2[:, s2, 1:2]), op=Alu.mult)
                    s4 = slice(e * X4, (e + 1) * X4)
                    nc.gpsimd.tensor_tensor(flat(v4[:, s4, 0:1]), flat(v4[:, s4, 0:1]),
                                            flat(v4[:, s4, 2:3]), op=Alu.mult)
                    s8 = slice(e * X8, (e + 1) * X8)
                    nc.gpsimd.tensor_tensor(flat(rh_v[:, s8, 0:1]), flat(rh_v[:, s8, 0:1]),
                                            flat(rh_v[:, s8, 4:5]), op=Alu.mult)
                    if e == 4:
                        # union gate only needs pu/pw; issue mid-stream so DVE
                        # reaches usage right as the last eighth finishes
                        nc.vector._custom_dve(ops["union_gate"], out=ug_full[:, :],
                                              in0=pu_full[:, :], in1=pw_full[:, :])
                ret_v = rh_v[:, :, 0:1].rearrange("p l o -> p (l o)")
                nc.vector.tensor_tensor(ug_full[:, :], ug_full[:, :], ret_v,
                                        op=Alu.mult)   # usage, in place
                nc.sync.dma_start(out=usf, in_=ug_full[:, :])

                # ---------- post-collective tail: quarter pipeline ----------
                with tc.high_priority():
                    Dg8 = cpool.tile([P, NCORES], F32)
                    nc.sync.dma_start(out=Dg8[:, :],
                                      in_=d_gathB.ap().broadcast_to([P, NCORES]))
                    Dg_bc = cpool.tile([P, 1], F32)
                    nc.vector.tensor_reduce(Dg_bc[:, :], Dg8[:, :], axis=AX, op=Alu.add)
                    B_bc = cpool.tile([P, 1], F32)
                    nc.vector.reciprocal(B_bc[:, :], Dg_bc[:, :])
                    nc.vector.tensor_tensor(B_bc[:, :], B_bc[:, :], ag_bc[:, :], op=Alu.mult)

                    ww_full = ssE  # dead after the diffs; reuse as scratch
                    # pre-arm the Act DGE queue during the collective so the
                    # first ww descriptor is at the head when its sem fires
                    nc.scalar.dma_start(out=d_scr.ap(), in_=DlB[:, :])
                    for q in range(2):
                        qs = slice(q * (L // 2), (q + 1) * (L // 2))
                        nc.scalar.activation(ww_full[:, qs], E_full[:, qs], Act.Copy,
                                             scale=B_bc[:, :])
                        nc.scalar.dma_start(out=wwf[:, qs], in_=ww_full[:, qs])
                        nc.vector.tensor_tensor(np_full[:, qs], np_full[:, qs],
                                                ww_full[:, qs], op=Alu.add)
                        nc.sync.dma_start(out=npf[:, qs], in_=np_full[:, qs])

    nc.compile()
    return nc


def _get_nc():
    if "nc" not in _CACHE:
        _CACHE["nc"] = _build()
    return _CACHE["nc"]


def _make_in_maps(inputs):
    mem = np.ascontiguousarray(inputs["memory"], dtype=np.float32)
    rw = np.ascontiguousarray(inputs["read_weighting"], dtype=np.float32)
    pu = np.ascontiguousarray(inputs["previous_usage"], dtype=np.float32)
    pw = np.ascontiguousarray(inputs["prev_write_weighting"], dtype=np.float32)
    prec = np.ascontiguousarray(inputs["precedence_weighting"], dtype=np.float32)
    wk = np.ascontiguousarray(inputs["write_key"], dtype=np.float32)
    fg = np.ascontiguousarray(inputs["free_gate"], dtype=np.float32)
    scal = np.array([inputs["write_strength"][0], inputs["allocation_gate"][0],
                     inputs["write_gate"][0]], dtype=np.float32)
    fgrep = np.tile(fg, FRW // RH)

    in_maps = []
    for c in range(NCORES):
        s = slice(c * R, (c + 1) * R)
        in_maps.append({
            "mem": mem[s], "rw": rw[s], "pu": pu[s], "pw": pw[s],
            "prec": prec[s], "wk": wk, "scal": scal, "fgrep": fgrep,
        })
    return in_maps


def _get_runner():
    """Jit the SPMD dispatch once per process; reuse across kernel() calls."""
    if "runner" in _CACHE:
        return _CACHE["runner"]
    import jax
    from jax.sharding import Mesh, PartitionSpec, NamedSharding
    from jax.experimental.shard_map import shard_map
    import concourse.mybir as mybir
    from concourse import bass2jax

    nc = _get_nc()
    bass2jax.install_neuronx_cc_hook()
    partition_name = nc.partition_id_tensor.name if nc.partition_id_tensor else None
    in_names, out_names, out_avals, zero_outs = [], [], [], []
    for alloc in nc.m.functions[0].allocations:
        if not isinstance(alloc, mybir.MemoryLocationSet):
            continue
        name = alloc.memorylocations[0].name
        if alloc.kind == "ExternalInput":
            if name != partition_name:
                in_names.append(name)
        elif alloc.kind == "ExternalOutput":
            shape = tuple(alloc.tensor_shape)
            dtype = mybir.dt.np(alloc.dtype)
            out_names.append(name)
            out_avals.append(jax.core.ShapedArray(shape, dtype))
            zero_outs.append(np.zeros(shape, dtype))
    n_params = len(in_names)
    all_in_names = list(in_names) + list(out_names)
    if partition_name is not None:
        all_in_names.append(partition_name)

    def _body(*args):
        operands = list(args)
        if partition_name is not None:
            operands.append(bass2jax.partition_id_tensor())
        return tuple(bass2jax._bass_exec_p.bind(
            *operands,
            out_avals=tuple(out_avals),
            in_names=tuple(all_in_names),
            out_names=tuple(out_names),
            lowering_input_output_aliases=(),
            sim_require_finite=True,
            sim_require_nnan=True,
            nc=nc,
        ))

    devices = jax.devices()[:NCORES]
    mesh = Mesh(np.asarray(devices), ("core",))
    in_specs = (PartitionSpec("core"),) * (n_params + len(out_names))
    out_specs = (PartitionSpec("core"),) * len(out_names)
    fn = jax.jit(shard_map(_body, mesh=mesh, in_specs=in_specs,
                           out_specs=out_specs, check_rep=False))
    sh = NamedSharding(mesh, PartitionSpec("core"))
    zeros_dev = [jax.device_put(
        np.zeros((NCORES * z.shape[0], *z.shape[1:]), z.dtype), sh)
        for z in zero_outs]

    def run(in_maps):
        concat_in = [np.concatenate(
            [np.asarray(in_maps[c][k]) for c in range(NCORES)], axis=0)
            for k in in_names]
        dev_in = [jax.device_put(a, sh) for a in concat_in]
        outs = fn(*dev_in, *zeros_dev)
        return {name: np.array(outs[i]) for i, name in enumerate(out_names)}

    _CACHE["runner"] = run
    return run


def _run_device(inputs):
    in_maps = _make_in_maps(inputs)
    try:
        out = _get_runner()(in_maps)
        return out["o_ww"], out["o_us"], out["o_np"]
    except Exception:
        from concourse.bass_utils import run_bass_kernel_spmd
        nc = _get_nc()
        res = run_bass_kernel_spmd(nc, in_maps, core_ids=list(range(NCORES)))
        ww = np.concatenate([res.results[c]["o_ww"] for c in range(NCORES)])
        us = np.concatenate([res.results[c]["o_us"] for c in range(NCORES)])
        npr = np.concatenate([res.results[c]["o_np"] for c in range(NCORES)])
        return ww, us, npr


def _alloc_fixup(usage, ww, npr, ag, wg):
    """Sparse allocation-weighting correction on the host (see module doc)."""
    K = 256
    while True:
        K = min(K, usage.shape[0])
        idx = np.argpartition(usage, K - 1)[:K]
        vals = usage[idx]
        srt = np.lexsort((idx, vals))   # stable: by value, then original index
        sv = vals[srt].astype(np.float32)
        si = idx[srt]
        cp = np.cumprod(sv, dtype=np.float32)
        if cp[-1] == 0.0 or K == usage.shape[0]:
            break
        K *= 4
    excl = np.empty_like(sv)
    excl[0] = np.float32(1.0)
    excl[1:] = cp[:-1]
    alloc = (np.float32(1.0) - sv) * excl
    nz = alloc != 0.0
    delta = np.float32(wg) * np.float32(ag) * alloc[nz]
    ww[si[nz]] += delta
    npr[si[nz]] += delta
    return ww, npr


def kernel(**inputs):
    ww, us, npr = _run_device(inputs)
    ag = float(np.float32(inputs["allocation_gate"][0]))
    wg = float(np.float32(inputs["write_gate"][0]))
    ww, npr = _alloc_fixup(us, ww, npr, ag, wg)
    return ww, us, npr
